# revision 12
# baseline (speedup 1.0000x reference)
"""Bass/Trainium2 kernel for nn_ConflictDetector (pairwise conflict scorer).

Reference computation:
    e  = concat(subj_emb, rel_emb, obj_emb) @ proj_w.T + proj_b        [N, 64]
    hi = e @ w1a.T ; hj = e @ w1b.T                                    [N, 64]
    h   = relu(hi[:,None,:] + hj[None,:,:] + b1)                       [N, N, 64]
    h2  = relu(h @ w2.T + b2)                                          [N, N, 32]
    s   = sigmoid(h2 @ w3[0] + b3[0])                                  [N, N]
    out = triu(s, k=1)

Strategy (data-parallel over pair rows, 8 cores):
  * Dedup claims on host (U ~1332 distinct of 2048); score the U x U grid
    of distinct claims on-device, gather back to [N, N] + triu on host.
  * Embedding + first linear run on host (tiny); device does the O(U^2)
    pairwise MLP.
  * Grid tiled into 64-row x 448-col units distributed round-robin over
    8 cores; per unit (16 quads of 4 i-rows):
      relu1: 2 ops per quad of [128,448] (4 i x 32 d partitions, one per
             d-half), bf16 in -> bf16 out, engine per R1_ENG knob.
             hj arrives 4x-replicated across partitions (hj4, host-packed)
             so a quad's 4 i-rows share one op per d-half.
      mm1  : 2 accumulating bf16 matmuls per quad (one per d-half), lhsT
             [128,128] = blockdiag over the 4 i's of w2^T's d-half ->
             h2 [128,448] f32 PSUM (2 quads per bank-aligned [128,1024]).
      relu2: 1 op per 2 quads of [128,896] PSUM -> fp8 SBUF (+b2), engine
             per R2_ENG.
      mm2  : 1 fp8 DoubleRow matmul per 2 quads accumulating [64,448]
             raw scores (fp8 rhs comes from relu2 for free).
      evac : copy PSUM -> bf16 SBUF, DMA out; sigmoid+b3 on host.
"""

import numpy as np
import ml_dtypes

N = 2048
D = 64
IB = 32      # i-block rows per unit (multiple of 8)
JW = 448     # j-width per unit
N_CORES = 8
SKIP_UNITS = True  # skip units where first_pos(row) >= last_pos(col) for all
BF16 = ml_dtypes.bfloat16
FP8 = ml_dtypes.float8_e4m3
NQUAD = IB // 4   # quads per unit
NGG = IB // 8     # group-pairs per unit
NR1 = IB // 2     # relu1 ops per unit

# Engine assignment knobs:
# R1_ENG[c]: engine for relu1 op c (0..31: quad g = c//2, d-half = c%2):
#   'v' = DVE tensor_scalar, 'a' = ACT activation, 'p' = GpSimd tensor_scalar
#   (GpSimd cannot access PSUM, so it can only serve relu1.)
# R2_ENG[gg]: engine for relu2 of group-pair gg (0..7) in every unit.
# EVAC_ENG: 'd' = DMA straight from PSUM (f32 out), else 'v'/'a' copy to bf16.
R1_ENG = "v" * NR1
R2_ENG = "a" * NGG
EVAC_ENG = "a"  # per-unit pattern, u % len
MM2_SWDR = False
# fp8 DoubleRow mm1: relu1 emits fp8, one DR matmul per quad (both d-halves
# contracted in one pass). Host prescales hj/cp/w2/w3 by PS so fp8 operands
# sit well inside e4m3's normal range; PSUM carries PS^2 * true values.
FP8_MM1 = True
PS = 16.0  # prescale factor

_CACHE = {}


def _build_bass(U):
    """U = units per core."""
    import concourse.bacc as bacc
    import concourse.mybir as mybir
    from concourse.tile import TileContext

    bf16 = mybir.dt.bfloat16
    fp8 = mybir.dt.float8e4
    f32 = mybir.dt.float32

    nc = bacc.Bacc(target_bir_lowering=False)

    # hj4: per d-half, hj rows 4x-replicated across partitions.
    hj_pack = nc.dram_tensor("hj_pack", [128, 2 * U * JW], bf16, kind="ExternalInput")
    # cw: packed weights [w2ds bf16-as-2-cols ... ] split into two tensors to
    # keep dtypes simple: w2 bf16 (fp8 when FP8_MM1), w3 fp8.
    w2_dt = fp8 if FP8_MM1 else bf16
    cw2 = nc.dram_tensor("cw2", [128, 256], w2_dt, kind="ExternalInput")
    cw3 = nc.dram_tensor("cw3", [128, NGG * 2 * IB], fp8, kind="ExternalInput")
    # cf0: b2p | unit-0 bias cols (tiny, leads the ring); cfr: the rest.
    cf0 = nc.dram_tensor("cf0", [128, 1 + NR1], f32, kind="ExternalInput")
    cfr = nc.dram_tensor(
        "cfr", [128, max(U - 1, 1) * NR1], f32, kind="ExternalInput"
    )
    out_dt = f32 if EVAC_ENG == "d" else bf16
    out = nc.dram_tensor("out", [U * IB, JW], out_dt, kind="ExternalOutput")

    add = mybir.AluOpType.add
    vmax = mybir.AluOpType.max
    Relu = mybir.ActivationFunctionType.Relu
    DR = (
        mybir.MatmulPerfMode.DoubleRowSwInterleave
        if MM2_SWDR
        else mybir.MatmulPerfMode.DoubleRow
    )

    with TileContext(nc) as tc:
        with (
            tc.tile_pool(name="const", bufs=1) as cpool,
            tc.tile_pool(name="rhs1", bufs=12) as rhs1pool,
            tc.tile_pool(name="rhs2", bufs=4) as rhs2pool,
            tc.tile_pool(name="sout", bufs=4) as soutpool,
            tc.tile_pool(name="ps1", bufs=3, space="PSUM") as ps1pool,
            tc.tile_pool(name="ps2", bufs=2, space="PSUM") as ps2pool,
        ):
            # Engine warm-ups with no DMA dependence: absorb dispatch-pipeline
            # latency during the preamble. (The ACT warm-up + table load are
            # emitted after the DMA issues below so they don't delay the DGEs.)
            warm = cpool.tile([128, 8], bf16)
            warm2 = cpool.tile([128, 8], bf16)
            nc.vector.memset(warm[:], 0.0)
            nc.vector.memset(warm2[:], 0.0)
            nc.tensor.ldweights(warm[:])
            # Dummy matmul with no DMA dependence: fills the PE/PSUM pipe so
            # the first real matmul doesn't pay first-use latency.
            warm_ps = ps1pool.tile([128, 1024], f32, name="h2_ps")
            nc.tensor.matmul(
                warm_ps[0:8, 0:8],
                lhsT=warm[:],
                rhs=warm2[:],
                start=True,
                stop=True,
            )
            # Per-unit hj tiles so unit-0 compute starts as soon as its own
            # slice lands: cf0 + hj0/hj1 lead the Sync ring, weights ride the
            # Scalar ring, and the bulk (hj2.., cfr) goes on the idle GpSimd
            # ring.
            cf0_sb = cpool.tile([128, 1 + NR1], f32)
            nc.sync.dma_start(out=cf0_sb[:], in_=cf0[:])
            hj_sbs = [
                cpool.tile([128, 2 * JW], bf16, name=f"hj_sb{u}") for u in range(U)
            ]
            # Critical first tiles spread across rings so they land in
            # parallel: unit-0 d-half 0 leads the scalar ring (gates the
            # first relu1), the tiny mm1 weights ride right behind it.
            nc.scalar.dma_start(out=hj_sbs[0][:, 0:JW], in_=hj_pack[:, 0:JW])
            # cw2 leads the GpSimd ring (tiny, lands before relu1 finishes)
            # so the first matmul's ring-count waits are scalar>=1 (hj0a,
            # already implied by its relu1 input) + gpsimd>=1 — satisfied the
            # moment its relu1 completes. cw3 (mm2, needed ~15us) follows;
            # all later GpSimd-ring waiters (cfr, hj bulk) have >2us slack.
            cw2_sb = cpool.tile([128, 256], w2_dt)
            nc.gpsimd.dma_start(out=cw2_sb[:], in_=cw2[:])
            nc.sync.dma_start(
                out=hj_sbs[0][:, JW : 2 * JW], in_=hj_pack[:, JW : 2 * JW]
            )
            cw3_sb = cpool.tile([128, NGG * 2 * IB], fp8)
            nc.gpsimd.dma_start(out=cw3_sb[:], in_=cw3[:])
            # ACT warm-up (pulls the Relu table load forward, after the DGEs).
            nc.scalar.activation(warm2[:], warm[:], Relu, bias=0.0, scale=1.0)
            b2p_sb = cf0_sb[:, 0:1]
            if U > 1:
                nc.sync.dma_start(
                    out=hj_sbs[1][:], in_=hj_pack[:, 2 * JW : 4 * JW]
                )
            cfr_sb = cpool.tile([128, max(U - 1, 1) * NR1], f32)
            nc.gpsimd.dma_start(out=cfr_sb[:], in_=cfr[:])
            for u in range(2, U):
                nc.gpsimd.dma_start(
                    out=hj_sbs[u][:],
                    in_=hj_pack[:, 2 * u * JW : 2 * (u + 1) * JW],
                )

            def cp_col(u, c):
                if u == 0:
                    return cf0_sb[:, 1 + c : 2 + c]
                return cfr_sb[:, (u - 1) * NR1 + c : (u - 1) * NR1 + c + 1]

            def eng(ch):
                return {"v": nc.vector, "a": nc.scalar, "p": nc.gpsimd}[ch]

            s_ps_of = {}
            pend_evac = {}

            def emit_gg(u, gg):
                if gg == 0:
                    s_ps_of[u] = ps2pool.tile([IB, JW], f32, name="s_ps")
                s_ps = s_ps_of[u]
                # Two quads (4 i's each) at bank-aligned 512-col slots.
                h2_ps = ps1pool.tile([128, 1024], f32)
                for g2 in range(2):
                    g = 2 * gg + g2  # quad index
                    rhs1 = rhs1pool.tile([128, 2 * JW], fp8 if FP8_MM1 else bf16)
                    for dh in range(2):
                        c = 2 * g + dh  # relu1 op index within unit
                        hj_u = hj_sbs[u][:, dh * JW : (dh + 1) * JW]
                        e = R1_ENG[c]
                        dst = rhs1[:, dh * JW : (dh + 1) * JW]
                        if e == "a":
                            nc.scalar.activation(
                                dst, hj_u, Relu, bias=cp_col(u, c), scale=1.0
                            )
                        else:
                            eng(e).tensor_scalar(
                                dst, hj_u, cp_col(u, c), 0.0, add, vmax
                            )
                        if not FP8_MM1:
                            nc.tensor.matmul(
                                h2_ps[:, g2 * 512 : g2 * 512 + JW],
                                lhsT=cw2_sb[:, dh * 128 : (dh + 1) * 128],
                                rhs=dst,
                                start=(dh == 0),
                                stop=(dh == 1),
                            )
                    if FP8_MM1:
                        # One DoubleRow matmul contracts both d-halves
                        # (virtual 256-deep) into this quad's h2.
                        nc.tensor.matmul(
                            h2_ps[:, g2 * 512 : g2 * 512 + JW],
                            lhsT=cw2_sb[:].rearrange("p (two m) -> p two m", two=2),
                            rhs=rhs1[:].rearrange("p (two j) -> p two j", two=2),
                            start=True,
                            stop=True,
                            perf_mode=DR,
                        )
                rhs2 = rhs2pool.tile([128, 2 * JW], fp8)
                h2_rd = h2_ps[:].rearrange("p (g j) -> p g j", g=2)[:, :, 0:JW]
                rhs2_wr = rhs2[:].rearrange("p (g j) -> p g j", g=2)
                e2 = R2_ENG[gg]
                if e2 == "a" or FP8_MM1:
                    # With FP8_MM1 the PSUM holds PS^2*(w2 h); ACT's scale
                    # rescales to PS*h2 so rhs2 stays in fp8 range.
                    nc.scalar.activation(
                        rhs2_wr,
                        h2_rd,
                        Relu,
                        bias=b2p_sb[:, 0:1],
                        scale=(1.0 / PS) if FP8_MM1 else 1.0,
                    )
                else:
                    eng(e2).tensor_scalar(
                        rhs2_wr, h2_rd, b2p_sb[:, 0:1], 0.0, add, vmax
                    )
                w3ap = cw3_sb[:, gg * 2 * IB : (gg + 1) * 2 * IB].rearrange(
                    "p (two f) -> p two f", two=2
                )
                nc.tensor.matmul(
                    s_ps[:],
                    lhsT=w3ap,
                    rhs=rhs2[:].rearrange("p (two j) -> p two j", two=2),
                    start=(gg == 0),
                    stop=(gg == NGG - 1),
                    perf_mode=DR,
                )

            def emit_evac(u):
                # Raw scores out; host applies sigmoid+b3.
                s_ps = s_ps_of.pop(u)
                ev = EVAC_ENG[u % len(EVAC_ENG)]
                if ev == "d":
                    # DMA straight from PSUM: no compute-engine time spent.
                    nc.sync.dma_start(
                        out=out[u * IB : (u + 1) * IB, :], in_=s_ps[:]
                    )
                    return
                # Last two units evacuate via DVE, which drains earlier.
                if u >= U - 2:
                    ev = "v"
                s_sb = soutpool.tile([IB, JW], bf16)
                if ev == "a":
                    nc.scalar.activation(
                        s_sb[:], s_ps[:], mybir.ActivationFunctionType.Copy
                    )
                else:
                    eng(ev).tensor_copy(out=s_sb[:], in_=s_ps[:])
                nc.sync.dma_start(out=out[u * IB : (u + 1) * IB, :], in_=s_sb[:])

            # Software-pipelined emission: the next unit's first group is
            # emitted before the current unit's last group, hiding the
            # s_ps evac round-trip at unit boundaries.
            sched = []
            for u in range(U):
                for gg in range(NGG):
                    sched.append((u, gg))
            if U > 1:
                for u in range(1, U):
                    i = sched.index((u - 1, NGG - 1))
                    sched[i], sched[i + 1] = sched[i + 1], sched[i]
            if U >= 2:
                # Fully interleave the last two units so the final unit's
                # chain isn't exposed serially at the end.
                tail = {(u, g) for u in (U - 2, U - 1) for g in range(NGG)}
                sched = [x for x in sched if x not in tail]
                for g in range(NGG):
                    sched.append((U - 2, g))
                    sched.append((U - 1, g))
            for u, gg in sched:
                emit_gg(u, gg)
                if gg == NGG - 1:
                    emit_evac(u)

    nc.finalize()
    return nc


def _get_nc(U):
    key = ("nc", U)
    if key not in _CACHE:
        _CACHE[key] = _build_bass(U)
    return _CACHE[key]


def kernel(
    subj_idx, rel_idx, obj_idx, subj_table, rel_table, obj_table,
    proj_w, proj_b, w1, b1, w2, b2, w3, b3,
):
    from concourse.bass_utils import run_bass_kernel_spmd

    subj_idx = np.asarray(subj_idx)
    rel_idx = np.asarray(rel_idx)
    obj_idx = np.asarray(obj_idx)
    subj_table = np.asarray(subj_table, np.float32)
    rel_table = np.asarray(rel_table, np.float32)
    obj_table = np.asarray(obj_table, np.float32)
    proj_w = np.asarray(proj_w, np.float32)
    proj_b = np.asarray(proj_b, np.float32)
    w1 = np.asarray(w1, np.float32)
    b1 = np.asarray(b1, np.float32)
    w2 = np.asarray(w2, np.float32)
    b2 = np.asarray(b2, np.float32)
    w3 = np.asarray(w3, np.float32)
    b3 = np.asarray(b3, np.float32)

    # ---- host: dedup claims ----
    key = (subj_idx.astype(np.int64) * rel_table.shape[0] + rel_idx) * obj_table.shape[
        0
    ] + obj_idx
    ukey, inv = np.unique(key, return_inverse=True)
    Uq = len(ukey)
    us = (ukey // (rel_table.shape[0] * obj_table.shape[0])).astype(np.int64)
    ur = ((ukey // obj_table.shape[0]) % rel_table.shape[0]).astype(np.int64)
    uo = (ukey % obj_table.shape[0]).astype(np.int64)

    # Entry (a, b) of the unique grid is needed only when
    # first_pos(a) < last_pos(b) (triu needs only position-ordered pairs).
    # Rows and columns are independent axes of the grid, so sort rows by
    # first-occurrence and columns by last-occurrence: both monotone, making
    # the needed-mask a staircase with maximal whole-unit skips.
    pos_first = np.full(Uq, N, np.int64)
    pos_last = np.full(Uq, -1, np.int64)
    np.minimum.at(pos_first, inv, np.arange(N))
    np.maximum.at(pos_last, inv, np.arange(N))
    if SKIP_UNITS:
        row_perm = np.argsort(pos_first, kind="stable")
        col_perm = np.argsort(pos_last, kind="stable")
    else:
        row_perm = col_perm = np.arange(Uq)
    rfirst = pos_first[row_perm]
    clast = pos_last[col_perm]
    row_rank = np.empty(Uq, np.int64)
    row_rank[row_perm] = np.arange(Uq)
    col_rank = np.empty(Uq, np.int64)
    col_rank[col_perm] = np.arange(Uq)

    n_ib = (Uq + IB - 1) // IB
    n_ju = (Uq + JW - 1) // JW
    units = [
        (b, j)
        for b in range(n_ib)
        for j in range(n_ju)
        if not SKIP_UNITS
        or rfirst[b * IB : min((b + 1) * IB, Uq)].min()
        < clast[j * JW : min((j + 1) * JW, Uq)].max()
    ]
    units_per_core = (len(units) + N_CORES - 1) // N_CORES
    n_slots = N_CORES * units_per_core
    units = units + [units[0]] * (n_slots - len(units))  # pad with dummies
    ipad = n_ib * IB
    jpad = n_ju * JW

    # ---- host: embedding + first linear for unique claims (tiny) ----
    combined = np.concatenate(
        [subj_table[us], rel_table[ur], obj_table[uo]], axis=-1
    )  # [Uq, 192]
    e = combined @ proj_w.T + proj_b  # [Uq, 64]
    w1a, w1b = w1[:, :D], w1[:, D:]
    hi = e @ w1a.T
    hj = e @ w1b.T
    ps = PS if FP8_MM1 else 1.0
    C = np.zeros((ipad, D), np.float32)
    C[:Uq] = (hi + b1)[row_perm] * ps  # per-row bias for relu1, row order
    hjT = np.zeros((D, jpad), np.float32)
    hjT[:, :Uq] = hj[col_perm].T * ps  # column order

    # ---- static packed weights (same for all cores) ----
    # w2ds [128, 2 d-halves, 128 outs]: matmul for d-half dh contracts
    # partitions p = 32q + r (q = quad member, r = d offset) holding
    # relu1(i_q, d=32*dh+r); out f = 32q+k gets w2[k, 32*dh+r].
    w2ds = np.zeros((128, 2, 128), np.float32)
    for q in range(4):
        for dh in range(2):
            w2ds[32 * q : 32 * (q + 1), dh, 32 * q : 32 * (q + 1)] = w2[
                :, 32 * dh : 32 * (dh + 1)
            ].T
    if FP8_MM1:
        cw2 = (w2ds * PS).reshape(128, 256).astype(FP8)
    else:
        cw2 = w2ds.reshape(128, 256).astype(BF16)

    # w3dr [128, 8 gg, 2 slots, 64 outs]: slot s of gg handles quad
    # g = 2*gg+s; member q (partitions 32q:32q+32 = its k dims) scores land
    # on out row 4g+q.
    w3dr = np.zeros((128, NGG, 2, IB), np.float32)
    for gg in range(NGG):
        for s in range(2):
            g = 2 * gg + s
            for q in range(4):
                w3dr[32 * q : 32 * (q + 1), gg, s, 4 * g + q] = w3[0]
    if MM2_SWDR:
        # stored[p, gg, 2t+s] = logical[p, gg, s, IB-1-t]
        w3dr = np.ascontiguousarray(
            np.transpose(w3dr[:, :, :, ::-1], (0, 1, 3, 2))
        )
    cw3 = (w3dr * ps).reshape(128, NGG * 2 * IB).astype(FP8)

    b2p = (np.tile(b2, 4) * ps).reshape(128, 1).astype(np.float32)

    # hj4 per d-half: rows 32*dh..32*dh+31 of hjT replicated 4x across
    # partitions (shared by a quad's 4 i-rows).
    hj4 = np.stack(
        [np.tile(hjT[32 * dh : 32 * (dh + 1), :], (4, 1)) for dh in range(2)]
    )  # [2, 128, jpad]

    # ---- per-core packs ----
    in_maps = []
    for c in range(N_CORES):
        units_c = units[c::N_CORES]
        hj_pack = np.zeros((128, units_per_core * 2 * JW), np.float32)
        cp_pack = np.zeros((128, units_per_core * NR1), np.float32)
        part_i = np.repeat(np.arange(4), 32)  # quad member per partition
        part_d = np.tile(np.arange(32), 4)
        for u, (b, ju) in enumerate(units_c):
            for dh in range(2):
                hj_pack[:, (2 * u + dh) * JW : (2 * u + dh + 1) * JW] = hj4[
                    dh, :, ju * JW : (ju + 1) * JW
                ]
            for g in range(NQUAD):
                for dh in range(2):
                    cc = 2 * g + dh
                    rows = IB * b + 4 * g + part_i  # [128] i row per partition
                    cp_pack[:, u * NR1 + cc] = C[rows, 32 * dh + part_d]
        cf0 = np.concatenate([b2p, cp_pack[:, :NR1]], axis=1)
        cfr = np.ascontiguousarray(cp_pack[:, NR1:])
        if cfr.shape[1] == 0:
            cfr = np.zeros((128, NR1), np.float32)
        in_maps.append(
            {
                "hj_pack": hj_pack.astype(BF16),
                "cw2": cw2,
                "cw3": cw3,
                "cf0": cf0,
                "cfr": cfr,
            }
        )

    nc = _get_nc(units_per_core)
    res = run_bass_kernel_spmd(
        nc, in_maps, core_ids=list(range(N_CORES)), **_CACHE.get("run_kwargs", {})
    )
    _CACHE["last_result"] = res

    # ---- gather: unit tiles -> unique grid -> full [N, N] -> triu ----
    ugrid = np.zeros((ipad, jpad), np.float32)
    seen = set()
    for c in range(N_CORES):
        units_c = units[c::N_CORES]
        out_c = res.results[c]["out"].reshape(units_per_core, IB, JW)
        for u, (b, ju) in enumerate(units_c):
            if (b, ju) in seen:
                continue  # dummy duplicate
            seen.add((b, ju))
            blk = out_c[u].astype(np.float64) / (ps * ps)
            # Scores leave the device pre-sigmoid; apply sigmoid+b3 here.
            blk = 1.0 / (1.0 + np.exp(-(blk + b3[0])))
            ugrid[b * IB : (b + 1) * IB, ju * JW : (ju + 1) * JW] = blk.astype(
                np.float32
            )
    scores = ugrid[np.ix_(row_rank[inv], col_rank[inv])]
    return np.triu(scores, k=1)

